# revision 16
# baseline (speedup 1.0000x reference)
"""Batched conv layer (im2col gather + einsum) as a Bass/Tile TRN2 kernel.

Problem: x (8,16,32,32,64) f32, kernel (8,3,3,64,128) f32
         out[b,i,oh,ow,f] = sum_{kh,kw,c} xpad[b,i,oh+kh-1,ow+kw-1,c] * kernel[b,kh,kw,c,f]
         out (8,16,32,32,128) f32
Sharding: batch dim b across 8 cores (pure data parallel, no collectives).

Per-core device layout (host prepares these):
  xp : (8 pairs, 128, 34*34) f16   partition dim packs 2 images x 64 channels;
                                   free dim is the zero-padded 34x34 image plane
  kd : (128, 9*128) f16            partition dim packs 2 copies of the 64 channels
                                   (one per image in a pair); free dim is
                                   9 taps x 128 output filters
  out: (16, 128, 1024) f16         [image, filter, position]; host transposes back
                                   and upcasts to f32 (fp16 store rounding is ~3e-4
                                   rel err, far inside the 2e-2 gate)

The conv is computed as 9 shifted matmuls accumulated in PSUM:
  out[f, pos] += ktap[c, f].T @ xwin[c, pos]   for each tap (kh, kw)
Images are processed in pairs occupying PE row-groups 0-63 / 64-127 so two
K=64 matmuls run concurrently in the 128x128 array.

Trace-driven layout (51.2us baseline):
  [5.8, 7.2]us  framework preamble (fixed)        [7.2, 10.2]us first input DMA chain
  [10.2, 44.3]us near-gapless matmul stream       [44.3, 48.8]us drain copies + stores
  [48.8, 57.0]us NRT postamble ladder (fixed ~8.3us)
Optimizations vs baseline: PE warm-up matmuls during the input-DMA window
(kills the 4us HAM cold-clock span), f16 stores (half the store wire time),
PSUM drains split across Vector and Scalar engines, per-image batched stores.
"""

import json
import os

import numpy as np

import concourse.bass as bass
import concourse.mybir as mybir
from concourse import bacc
from concourse.bass_utils import run_bass_kernel_spmd
from concourse.tile import TileContext

# Static problem config (hardcoded per the harness contract)
B, I, H, W, C, F = 8, 16, 32, 32, 64, 128
KD = 3
HP = H + 2  # padded
WP = W + 2
NPOS = H * W          # 1024 output positions per image
NTILE = 512           # positions per PSUM tile (one bank)
NHALF = NPOS // NTILE  # 2
ROWS_PER_TILE = NTILE // W  # 16 output rows per tile
N_CORES = 8

# matmul input dtype: "f16" (default: ~3e-4 rel err, fastest), "f32r"
# (~1.5e-4), "f32" (exact, 4x slower PE), "bf16"
MM_DTYPE = os.environ.get("CONV_MM_DTYPE", "f16")
# weight (stationary operand) dtype: "" = same as MM_DTYPE
W_DTYPE = os.environ.get("CONV_W_DTYPE", "")
# Number of PE warm-up matmuls (0 disables). Each is a K=128 N=256 f16 matmul
# on a memset tile; at the cold 1.2 GHz clock ~14 of them cover the ~3.5us
# HAM activity window while the first input DMAs are still in flight.
WARMUP_N = int(os.environ.get("CONV_WARMUP_N", "12"))
# Drop InstLdweights whose weights are already resident in the target PE
# row-group (schedule pairs same-weight matmuls back to back, so half of the
# body's weight loads are redundant).
DEDUPE_LDW = os.environ.get("CONV_DEDUPE_LDW", "1") == "1"
# Store outputs as f16 (host upcasts) instead of f32.
F16_OUT = os.environ.get("CONV_F16_OUT", "1") == "1"

_CACHED_NC = None
LAST_RESULTS = None


def _dedupe_ldweights(nc):
    """Drop InstLdweights whose weights are already resident in the PE array.

    The Tile legalizer emits one InstLdweights per matmul (all matmuls are
    ldweights=False, i.e. non-self-loading).  The schedule places the two
    half-tiles of one (tap, row-group) back to back, so every second load of
    a row-group re-loads identical weights.  Each PE row-group (64-row band
    here) holds its stationary operand independently, so the second load is
    redundant: removing it keeps the matmuls correct and halves the
    weight-load pressure on the PE's weight path.

    Dropped waits are safe: an identical (sem >= value) wait was already
    enforced by the retained load earlier on the same strict-FIFO PE queue,
    and semaphores only count up within a run.
    """
    total_removed = 0
    for bb in nc.main_func.blocks:
        removed = 0
        insts = list(bb.instructions)
        out = []
        resident = {}  # (base_partition, row_span) -> weights key
        for inst in insts:
            nm = type(inst).__name__
            if nm == 'InstLdweights':
                js = json.loads(nc.instruction_to_json(inst))
                w = js['ins'][0]
                tp = js.get('tile_position') or [0, 0]
                ts = js.get('tile_size') or [128, 128]
                base, span = tp[0], ts[0]
                key = (w.get('memref'), w.get('offset'), str(w.get('ap')),
                       str(ts), str(js.get('perf_mode')))
                si = js.get('sync_info') or {}
                if resident.get((base, span)) == key and not si.get('on_update'):
                    removed += 1
                    continue
                # invalidate any tracked band overlapping [base, base+span)
                for (b, s) in list(resident):
                    if b < base + span and base < b + s and (b, s) != (base, span):
                        del resident[(b, s)]
                resident[(base, span)] = key
            elif nm == 'InstMatmult':
                pass  # matmuls only consume resident weights
            else:
                # Any other PE-queue instruction leaves the array untouched;
                # non-PE instructions are irrelevant.  Keep tracking.
                pass
            out.append(inst)
        if removed:
            bb.instructions = out
            total_removed += removed
    return total_removed


def _build_nc():
    nc = bacc.Bacc(trn_type="TRN2")

    mm_dt = {
        "f32": mybir.dt.float32,
        "f32r": mybir.dt.float32r,
        "bf16": mybir.dt.bfloat16,
        "f16": mybir.dt.float16,
    }[MM_DTYPE]
    # For f32r, type the DRAM inputs as float32r end-to-end (same 4-byte fp32
    # layout; the PE just reads fewer mantissa bits) so the BIR verifier sees a
    # consistent fp32r producer chain.  For f16 the host pre-casts the inputs.
    if MM_DTYPE in ("f32r", "f16"):
        in_dt = mm_dt
    else:
        in_dt = mybir.dt.float32

    k_dt = mybir.dt.float16 if W_DTYPE == "f16" else in_dt
    out_dt = mybir.dt.float16 if F16_OUT else mybir.dt.float32

    xp = nc.declare_dram_parameter("xp", [I // 2, 128, HP * WP], in_dt, isOutput=False)
    kd = nc.declare_dram_parameter("kd", [128, KD * KD * F], k_dt, isOutput=False)
    out = nc.declare_dram_parameter("out", [I, F, NPOS], out_dt, isOutput=True)

    with TileContext(nc) as tc:
        with (
            tc.tile_pool(name="kpool", bufs=1) as kpool,
            tc.tile_pool(name="xpool", bufs=8) as xpool,
            tc.tile_pool(name="opool", bufs=16) as opool,
            tc.tile_pool(name="psum", bufs=8, space="PSUM") as psum_pool,
        ):
            # PE warm-up: the HAM clock gate runs the PE at 1.2 GHz until it
            # has seen ~3.4us of sustained activity.  Burn dummy matmuls
            # while the input DMAs are in flight so the real matmuls (gated
            # on the first input DMA completion) run at 2.4 GHz as early as
            # possible.  The source is a *raw* (non-Tile) SBUF tensor so the
            # warm-up carries no dependencies and starts right after the
            # framework preamble; whatever bytes it holds are multiplied and
            # discarded.  Sized so the warm-up queue drains just before the
            # first input DMA's completion semaphore fires - warm-up matmuls
            # past that point would push the real stream back 1:1, while a
            # slightly-early finish merely leaves a few half-clock real
            # matmuls.  The warm-up PSUM bank is the first rotation slot of
            # the main pool, recycled for real accumulation chains after.
            if WARMUP_N > 0:
                wsrc = nc.alloc_sbuf_tensor("warm_src", [128, 384],
                                            mybir.dt.float16)
                wpsum = psum_pool.tile([128, NTILE], mybir.dt.float32, tag="ps",
                                       name="warm_ps")
                for i in range(WARMUP_N):
                    o0 = (i % 2) * 256
                    nc.tensor.matmul(
                        wpsum[:, o0:o0 + 256],
                        wsrc.ap()[:, 0:128],
                        wsrc.ap()[:, 128:384],
                        start=True, stop=True, skip_group_check=True,
                    )

            # Loads go on the Scalar-engine HWDGE queue, stores on the Sync
            # queue - two independent FIFOs so the output stores never
            # serialize behind input loads.
            load_eng = nc.gpsimd if MM_DTYPE == "bf16" else nc.scalar

            x_dt = mybir.dt.bfloat16 if MM_DTYPE == "bf16" else in_dt

            # Pair 0 first.  Its first 18 padded rows (all that half 0's nine
            # taps touch) are split across BOTH HWDGE queues so wire time and
            # the ~1.5us HBM completion receipt are paid in parallel; tap-0
            # weights ride the otherwise-idle GpSimd SWDGE queue.  This puts
            # the first real matmul's dependencies ~0.4us earlier than a
            # single-queue load.
            xtiles = []
            xtile0 = xpool.tile([128, HP, WP], x_dt, tag="x")
            ktile = kpool.tile([128, KD * KD, F],
                               mybir.dt.bfloat16 if MM_DTYPE == "bf16" else k_dt)
            if MM_DTYPE == "bf16":
                nc.scalar.dma_start(out=xtile0[:, 0:18, :].rearrange("p h w -> p (h w)"),
                                    in_=xp[0, :, 0:18 * WP])
                nc.gpsimd.dma_start(out=ktile.rearrange("p t f -> p (t f)"), in_=kd[:, :])
                nc.gpsimd.dma_start(out=xtile0[:, 18:HP, :].rearrange("p h w -> p (h w)"),
                                    in_=xp[0, :, 18 * WP:HP * WP])
            else:
                nc.scalar.dma_start(out=xtile0[:, 0:9, :].rearrange("p h w -> p (h w)"),
                                    in_=xp[0, :, 0:9 * WP])
                nc.sync.dma_start(out=xtile0[:, 9:18, :].rearrange("p h w -> p (h w)"),
                                  in_=xp[0, :, 9 * WP:18 * WP])
                nc.gpsimd.dma_start(out=ktile[:, 0, :], in_=kd[:, 0:F])
                nc.sync.dma_start(
                    out=ktile[:, 1:KD * KD, :].rearrange("p t f -> p (t f)"),
                    in_=kd[:, F:KD * KD * F])
                load_eng.dma_start(out=xtile0[:, 18:HP, :].rearrange("p h w -> p (h w)"),
                                   in_=xp[0, :, 18 * WP:HP * WP])
            xtiles.append(xtile0)

            for pair in range(1, I // 2):
                xt = xpool.tile([128, HP, WP], x_dt, name=f"x_{pair}", tag="x")
                load_eng.dma_start(out=xt.rearrange("p h w -> p (h w)"), in_=xp[pair])
                xtiles.append(xt)

            def emit_mm(psums, xtile, schedule):
                # schedule: list of (half, par, t)
                for half, par, t in schedule:
                    kh, kw = divmod(t, KD)
                    oh0 = half * ROWS_PER_TILE
                    p0 = par * 64
                    lhsT = ktile[p0:p0 + 64, t, :]
                    rhs = xtile[p0:p0 + 64, oh0 + kh:oh0 + kh + ROWS_PER_TILE,
                                kw:kw + W]
                    nc.tensor.matmul(
                        psums[half][par][:, :], lhsT, rhs,
                        start=(t == 0), stop=(t == KD * KD - 1),
                    )

            def vec_copy(out, in_):
                nc.vector.tensor_copy(out=out, in_=in_)

            def act_copy(out, in_):
                # ScalarE has no tensor_copy; activation(Copy) is its copy op
                # (and it reads PSUM faster than DVE does).
                nc.scalar.activation(out, in_, mybir.ActivationFunctionType.Copy)

            copy_engines = (vec_copy, act_copy)

            for pair in range(I // 2):
                xtile = xtiles[pair]
                psums = []
                for half in range(NHALF):
                    row = []
                    for par in range(2):
                        ps = psum_pool.tile([128, NTILE], mybir.dt.float32,
                                            name=f"ps_{pair}_{half}_{par}", tag="ps")
                        row.append(ps)
                    psums.append(row)

                if pair == 0:
                    # half-major: half 0 only needs the first row-split load
                    sched = [(h, par, t) for h in range(NHALF)
                             for t in range(KD * KD) for par in range(2)]
                elif pair == I // 2 - 1:
                    # Last pair: taps 0-5 tap-major, then each chain finishes
                    # its last 3 taps as a trio.  Chains alternate PE
                    # row-groups so consecutive trios still overlap, but the
                    # completions spread ~0.6us apart - the PSUM drain copies
                    # then run under the trailing matmuls instead of
                    # serializing after the final one.
                    sched = [(h, par, t) for t in range(KD * KD - 4)
                             for par in range(2) for h in range(NHALF)]
                    for h, par in ((0, 0), (0, 1), (1, 0), (1, 1)):
                        sched += [(h, par, t) for t in range(KD * KD - 4, KD * KD)]
                else:
                    # Tap-major with both halves of a row-group adjacent:
                    # per tap, (h0,p0),(h1,p0),(h0,p1),(h1,p1).  The two
                    # same-par matmuls share identical weights, so the LDW
                    # dedupe pass drops the second load; the other par's LDW
                    # prefetches into the opposite row-group during the
                    # current par's two matmuls.  Chain completions at the
                    # last tap land ~1 slot apart; the Vector/Scalar copy
                    # split absorbs that reduced stagger.
                    sched = [(h, par, t) for t in range(KD * KD)
                             for par in range(2) for h in range(NHALF)]
                emit_mm(psums, xtile, sched)

                # Drain both halves of each image into one [128, NPOS] f16
                # tile, then store the whole image in a single DMA.  The two
                # images of a pair drain on different engines (Vector / Scalar
                # - both can read PSUM, different banks) so the four copies
                # finish in two copy-times instead of four.  Stores get the
                # same affinity: even images ride the Sync HWDGE queue, odd
                # images the Scalar HWDGE queue (free once the input loads
                # have issued) - a DMA_DIRECT2D occupies its queue ~600 ns,
                # so splitting halves the store-issue serialization and the
                # final stores of the last pair go out two-at-a-time.
                last_pair = pair == I // 2 - 1
                store_engines = (nc.sync, nc.scalar)
                for par in range(2):
                    i_img = pair * 2 + par
                    cp = copy_engines[par]
                    st = store_engines[par]
                    otile = opool.tile([128, NPOS], out_dt,
                                       name=f"o_{pair}_{par}", tag="o")
                    for half in range(NHALF):
                        cp(otile[:, half * NTILE:(half + 1) * NTILE],
                           psums[half][par][:, :])
                        if last_pair:
                            # Final pair: store per-half right after each copy
                            # so the very last DMA is small and its completion
                            # receipt lands sooner.
                            st.dma_start(
                                out=out[i_img, :, half * NTILE:(half + 1) * NTILE],
                                in_=otile[:, half * NTILE:(half + 1) * NTILE])
                    if not last_pair:
                        st.dma_start(out=out[i_img, :, :], in_=otile[:, :])
    if DEDUPE_LDW:
        _dedupe_ldweights(nc)
    nc.compile()
    return nc


def _prep_core_inputs(x_b: np.ndarray, k_b: np.ndarray):
    """x_b (16,32,32,64) f32, k_b (3,3,64,128) f32 -> device layouts."""
    np_in = np.float16 if MM_DTYPE == "f16" else np.float32
    xpad = np.zeros((I, HP, WP, C), dtype=np_in)
    xpad[:, 1:H + 1, 1:W + 1, :] = x_b
    # (I, HP, WP, C) -> (I, C, HP, WP) -> (I//2, 2*C, HP*WP)
    xp = np.ascontiguousarray(xpad.transpose(0, 3, 1, 2)).reshape(I // 2, 2 * C, HP * WP)

    kc = k_b.reshape(KD * KD, C, F)                       # (9, 64, 128)
    kdup = np.concatenate([kc, kc], axis=1)               # (9, 128, 128)
    kd = np.ascontiguousarray(kdup.transpose(1, 0, 2)).reshape(128, KD * KD * F)
    if W_DTYPE == "f16" or MM_DTYPE == "f16":
        kd = kd.astype(np.float16)
    return {"xp": xp, "kd": kd}


def kernel(**inputs) -> np.ndarray:
    global _CACHED_NC, LAST_RESULTS
    x = np.asarray(inputs["x"], dtype=np.float32)
    k = np.asarray(inputs["kernel"], dtype=np.float32)

    if _CACHED_NC is None:
        _CACHED_NC = _build_nc()
    nc = _CACHED_NC

    in_maps = [_prep_core_inputs(x[b], k[b]) for b in range(B)]
    res = run_bass_kernel_spmd(nc, in_maps, core_ids=list(range(N_CORES)))
    LAST_RESULTS = res

    outs = []
    for b in range(B):
        o = res.results[b]["out"]                          # (16, 128, 1024)
        o = o.transpose(0, 2, 1).reshape(I, H, W, F)       # (16, 32, 32, 128)
        outs.append(o)
    return np.ascontiguousarray(np.stack(outs, axis=0).astype(np.float32))


# revision 19
# speedup vs baseline: 1.0185x; 1.0185x over previous
"""Batched conv layer (im2col gather + einsum) as a Bass/Tile TRN2 kernel.

Problem: x (8,16,32,32,64) f32, kernel (8,3,3,64,128) f32
         out[b,i,oh,ow,f] = sum_{kh,kw,c} xpad[b,i,oh+kh-1,ow+kw-1,c] * kernel[b,kh,kw,c,f]
         out (8,16,32,32,128) f32
Sharding: batch dim b across 8 cores (pure data parallel, no collectives).

Per-core device layout (host prepares these):
  xp : (8 pairs, 128, 34*34) f16   partition dim packs 2 images x 64 channels;
                                   free dim is the zero-padded 34x34 image plane
  kd : (128, 9*128) f16            partition dim packs 2 copies of the 64 channels
                                   (one per image in a pair); free dim is
                                   9 taps x 128 output filters
  out: (16, 128, 1024) f16         [image, filter, position]; host transposes back
                                   and upcasts to f32 (fp16 store rounding is ~3e-4
                                   rel err, far inside the 2e-2 gate)

The conv is computed as 9 shifted matmuls accumulated in PSUM:
  out[f, pos] += ktap[c, f].T @ xwin[c, pos]   for each tap (kh, kw)
Images are processed in pairs occupying PE row-groups 0-63 / 64-127 so two
K=64 matmuls run concurrently in the 128x128 array.

Trace-driven layout (51.2us baseline):
  [5.8, 7.2]us  framework preamble (fixed)        [7.2, 10.2]us first input DMA chain
  [10.2, 44.3]us near-gapless matmul stream       [44.3, 48.8]us drain copies + stores
  [48.8, 57.0]us NRT postamble ladder (fixed ~8.3us)
Optimizations vs baseline: PE warm-up matmuls during the input-DMA window
(kills the 4us HAM cold-clock span), f16 stores (half the store wire time),
PSUM drains split across Vector and Scalar engines, per-image batched stores.
"""

import json
import os

import numpy as np

import concourse.bass as bass
import concourse.mybir as mybir
from concourse import bacc
from concourse.bass_utils import run_bass_kernel_spmd
from concourse.tile import TileContext

# Static problem config (hardcoded per the harness contract)
B, I, H, W, C, F = 8, 16, 32, 32, 64, 128
KD = 3
HP = H + 2  # padded
WP = W + 2
NPOS = H * W          # 1024 output positions per image
NTILE = 512           # positions per PSUM tile (one bank)
NHALF = NPOS // NTILE  # 2
ROWS_PER_TILE = NTILE // W  # 16 output rows per tile
N_CORES = 8

# matmul input dtype: "f16" (default: ~3e-4 rel err, fastest), "f32r"
# (~1.5e-4), "f32" (exact, 4x slower PE), "bf16"
MM_DTYPE = os.environ.get("CONV_MM_DTYPE", "f16")
# weight (stationary operand) dtype: "" = same as MM_DTYPE
W_DTYPE = os.environ.get("CONV_W_DTYPE", "")
# Number of PE warm-up matmuls (0 disables). Each is a K=128 N=256 f16 matmul
# on a memset tile; at the cold 1.2 GHz clock ~14 of them cover the ~3.5us
# HAM activity window while the first input DMAs are still in flight.
WARMUP_N = int(os.environ.get("CONV_WARMUP_N", "17"))
# Drop InstLdweights whose weights are already resident in the target PE
# row-group (schedule pairs same-weight matmuls back to back, so half of the
# body's weight loads are redundant).
DEDUPE_LDW = os.environ.get("CONV_DEDUPE_LDW", "1") == "1"
# Store outputs as f16 (host upcasts) instead of f32.
F16_OUT = os.environ.get("CONV_F16_OUT", "1") == "1"

_CACHED_NC = None
LAST_RESULTS = None


def _dedupe_ldweights(nc):
    """Drop InstLdweights whose weights are already resident in the PE array.

    The Tile legalizer emits one InstLdweights per matmul (all matmuls are
    ldweights=False, i.e. non-self-loading).  The schedule places the two
    half-tiles of one (tap, row-group) back to back, so every second load of
    a row-group re-loads identical weights.  Each PE row-group (64-row band
    here) holds its stationary operand independently, so the second load is
    redundant: removing it keeps the matmuls correct and halves the
    weight-load pressure on the PE's weight path.

    Dropped waits are safe: an identical (sem >= value) wait was already
    enforced by the retained load earlier on the same strict-FIFO PE queue,
    and semaphores only count up within a run.
    """
    total_removed = 0
    for bb in nc.main_func.blocks:
        removed = 0
        insts = list(bb.instructions)
        out = []
        resident = {}  # (base_partition, row_span) -> weights key
        for inst in insts:
            nm = type(inst).__name__
            if nm == 'InstLdweights':
                js = json.loads(nc.instruction_to_json(inst))
                w = js['ins'][0]
                tp = js.get('tile_position') or [0, 0]
                ts = js.get('tile_size') or [128, 128]
                base, span = tp[0], ts[0]
                key = (w.get('memref'), w.get('offset'), str(w.get('ap')),
                       str(ts), str(js.get('perf_mode')))
                si = js.get('sync_info') or {}
                if resident.get((base, span)) == key and not si.get('on_update'):
                    removed += 1
                    continue
                # invalidate any tracked band overlapping [base, base+span)
                for (b, s) in list(resident):
                    if b < base + span and base < b + s and (b, s) != (base, span):
                        del resident[(b, s)]
                resident[(base, span)] = key
            elif nm == 'InstMatmult':
                pass  # matmuls only consume resident weights
            else:
                # Any other PE-queue instruction leaves the array untouched;
                # non-PE instructions are irrelevant.  Keep tracking.
                pass
            out.append(inst)
        if removed:
            bb.instructions = out
            total_removed += removed
    return total_removed


def _build_nc():
    nc = bacc.Bacc(trn_type="TRN2")

    mm_dt = {
        "f32": mybir.dt.float32,
        "f32r": mybir.dt.float32r,
        "bf16": mybir.dt.bfloat16,
        "f16": mybir.dt.float16,
    }[MM_DTYPE]
    # For f32r, type the DRAM inputs as float32r end-to-end (same 4-byte fp32
    # layout; the PE just reads fewer mantissa bits) so the BIR verifier sees a
    # consistent fp32r producer chain.  For f16 the host pre-casts the inputs.
    if MM_DTYPE in ("f32r", "f16"):
        in_dt = mm_dt
    else:
        in_dt = mybir.dt.float32

    k_dt = mybir.dt.float16 if W_DTYPE == "f16" else in_dt
    out_dt = mybir.dt.float16 if F16_OUT else mybir.dt.float32

    xp = nc.declare_dram_parameter("xp", [I // 2, 128, HP * WP], in_dt, isOutput=False)
    kd = nc.declare_dram_parameter("kd", [128, KD * KD * F], k_dt, isOutput=False)
    out = nc.declare_dram_parameter("out", [I, F, NPOS], out_dt, isOutput=True)

    with TileContext(nc) as tc:
        with (
            tc.tile_pool(name="kpool", bufs=1) as kpool,
            tc.tile_pool(name="xpool", bufs=8) as xpool,
            tc.tile_pool(name="opool", bufs=16) as opool,
            tc.tile_pool(name="psum", bufs=8, space="PSUM") as psum_pool,
        ):
            # PE warm-up: the HAM clock gate runs the PE at 1.2 GHz until it
            # has seen ~3.4us of sustained activity.  Burn dummy matmuls
            # while the input DMAs are in flight so the real matmuls (gated
            # on the first input DMA completion) run at 2.4 GHz as early as
            # possible.  The source is a *raw* (non-Tile) SBUF tensor so the
            # warm-up carries no dependencies and starts right after the
            # framework preamble; whatever bytes it holds are multiplied and
            # discarded.  Sized so the warm-up queue drains just before the
            # first input DMA's completion semaphore fires - warm-up matmuls
            # past that point would push the real stream back 1:1, while a
            # slightly-early finish merely leaves a few half-clock real
            # matmuls.  The warm-up PSUM bank is the first rotation slot of
            # the main pool, recycled for real accumulation chains after.
            if WARMUP_N > 0:
                wsrc = nc.alloc_sbuf_tensor("warm_src", [128, 384],
                                            mybir.dt.float16)
                wpsum = psum_pool.tile([128, NTILE], mybir.dt.float32, tag="ps",
                                       name="warm_ps")
                for i in range(WARMUP_N):
                    o0 = (i % 2) * 256
                    nc.tensor.matmul(
                        wpsum[:, o0:o0 + 256],
                        wsrc.ap()[:, 0:128],
                        wsrc.ap()[:, 128:384],
                        start=True, stop=True, skip_group_check=True,
                    )

            # Loads go on the Scalar-engine HWDGE queue, stores on the Sync
            # queue - two independent FIFOs so the output stores never
            # serialize behind input loads.
            load_eng = nc.gpsimd if MM_DTYPE == "bf16" else nc.scalar

            x_dt = mybir.dt.bfloat16 if MM_DTYPE == "bf16" else in_dt

            # Pair 0 first.  Its first 18 padded rows (all that half 0's nine
            # taps touch) are split across BOTH HWDGE queues so wire time and
            # the ~1.5us HBM completion receipt are paid in parallel; tap-0
            # weights ride the otherwise-idle GpSimd SWDGE queue.  This puts
            # the first real matmul's dependencies ~0.4us earlier than a
            # single-queue load.
            xtiles = []
            xtile0 = xpool.tile([128, HP, WP], x_dt, tag="x")
            ktile = kpool.tile([128, KD * KD, F],
                               mybir.dt.bfloat16 if MM_DTYPE == "bf16" else k_dt)
            if MM_DTYPE == "bf16":
                nc.scalar.dma_start(out=xtile0[:, 0:18, :].rearrange("p h w -> p (h w)"),
                                    in_=xp[0, :, 0:18 * WP])
                nc.gpsimd.dma_start(out=ktile.rearrange("p t f -> p (t f)"), in_=kd[:, :])
                nc.gpsimd.dma_start(out=xtile0[:, 18:HP, :].rearrange("p h w -> p (h w)"),
                                    in_=xp[0, :, 18 * WP:HP * WP])
            else:
                # Both HWDGE queues open with the first matmul's dependencies
                # (tap-0 weights are tiny, then a 78 KB x half each); the
                # remaining kernel taps follow in two chunks sized so each
                # tap's weights land just ahead of pair-0's half-major tap
                # progression.
                nc.scalar.dma_start(out=ktile[:, 0, :], in_=kd[:, 0:F])
                nc.sync.dma_start(out=xtile0[:, 9:18, :].rearrange("p h w -> p (h w)"),
                                  in_=xp[0, :, 9 * WP:18 * WP])
                nc.scalar.dma_start(out=xtile0[:, 0:9, :].rearrange("p h w -> p (h w)"),
                                    in_=xp[0, :, 0:9 * WP])
                nc.sync.dma_start(
                    out=ktile[:, 1:3, :].rearrange("p t f -> p (t f)"),
                    in_=kd[:, F:3 * F])
                nc.sync.dma_start(
                    out=ktile[:, 3:KD * KD, :].rearrange("p t f -> p (t f)"),
                    in_=kd[:, 3 * F:KD * KD * F])
                load_eng.dma_start(out=xtile0[:, 18:HP, :].rearrange("p h w -> p (h w)"),
                                   in_=xp[0, :, 18 * WP:HP * WP])
            xtiles.append(xtile0)

            for pair in range(1, I // 2):
                xt = xpool.tile([128, HP, WP], x_dt, name=f"x_{pair}", tag="x")
                load_eng.dma_start(out=xt.rearrange("p h w -> p (h w)"), in_=xp[pair])
                xtiles.append(xt)

            def emit_mm(psums, xtile, schedule):
                # schedule: list of (half, par, t)
                for half, par, t in schedule:
                    kh, kw = divmod(t, KD)
                    oh0 = half * ROWS_PER_TILE
                    p0 = par * 64
                    lhsT = ktile[p0:p0 + 64, t, :]
                    rhs = xtile[p0:p0 + 64, oh0 + kh:oh0 + kh + ROWS_PER_TILE,
                                kw:kw + W]
                    nc.tensor.matmul(
                        psums[half][par][:, :], lhsT, rhs,
                        start=(t == 0), stop=(t == KD * KD - 1),
                    )

            def vec_copy(out, in_):
                nc.vector.tensor_copy(out=out, in_=in_)

            def act_copy(out, in_):
                # ScalarE has no tensor_copy; activation(Copy) is its copy op
                # (and it reads PSUM faster than DVE does).
                nc.scalar.activation(out, in_, mybir.ActivationFunctionType.Copy)

            copy_engines = (vec_copy, act_copy)

            for pair in range(I // 2):
                xtile = xtiles[pair]
                psums = []
                for half in range(NHALF):
                    row = []
                    for par in range(2):
                        ps = psum_pool.tile([128, NTILE], mybir.dt.float32,
                                            name=f"ps_{pair}_{half}_{par}", tag="ps")
                        row.append(ps)
                    psums.append(row)

                if pair == 0:
                    # half-major: half 0 only needs the first row-split load
                    sched = [(h, par, t) for h in range(NHALF)
                             for t in range(KD * KD) for par in range(2)]
                elif pair == I // 2 - 1:
                    # Last pair: taps 0-5 tap-major, then each chain finishes
                    # its last 3 taps as a trio.  Chains alternate PE
                    # row-groups so consecutive trios still overlap, but the
                    # completions spread ~0.6us apart - the PSUM drain copies
                    # then run under the trailing matmuls instead of
                    # serializing after the final one.
                    sched = [(h, par, t) for t in range(KD * KD - 4)
                             for par in range(2) for h in range(NHALF)]
                    for h, par in ((0, 0), (0, 1), (1, 0), (1, 1)):
                        sched += [(h, par, t) for t in range(KD * KD - 4, KD * KD)]
                else:
                    # Tap-major with both halves of a row-group adjacent:
                    # per tap, (h0,p0),(h1,p0),(h0,p1),(h1,p1).  The two
                    # same-par matmuls share identical weights, so the LDW
                    # dedupe pass drops the second load; the other par's LDW
                    # prefetches into the opposite row-group during the
                    # current par's two matmuls.  Chain completions at the
                    # last tap land ~1 slot apart; the Vector/Scalar copy
                    # split absorbs that reduced stagger.
                    sched = [(h, par, t) for t in range(KD * KD)
                             for par in range(2) for h in range(NHALF)]
                emit_mm(psums, xtile, sched)

                # Drain both halves of each image into one [128, NPOS] f16
                # tile, then store the whole image in a single DMA.  The two
                # images of a pair drain on different engines (Vector / Scalar
                # - both can read PSUM, different banks) so the four copies
                # finish in two copy-times instead of four.  Stores get the
                # same affinity: even images ride the Sync HWDGE queue, odd
                # images the Scalar HWDGE queue (free once the input loads
                # have issued) - a DMA_DIRECT2D occupies its queue ~600 ns,
                # so splitting halves the store-issue serialization and the
                # final stores of the last pair go out two-at-a-time.
                last_pair = pair == I // 2 - 1
                store_engines = (nc.sync, nc.scalar)
                for par in range(2):
                    i_img = pair * 2 + par
                    cp = copy_engines[par]
                    st = store_engines[par]
                    otile = opool.tile([128, NPOS], out_dt,
                                       name=f"o_{pair}_{par}", tag="o")
                    for half in range(NHALF):
                        cp(otile[:, half * NTILE:(half + 1) * NTILE],
                           psums[half][par][:, :])
                        if last_pair:
                            # Final pair: store per-half right after each copy
                            # so the very last DMA is small and its completion
                            # receipt lands sooner.
                            st.dma_start(
                                out=out[i_img, :, half * NTILE:(half + 1) * NTILE],
                                in_=otile[:, half * NTILE:(half + 1) * NTILE])
                    if not last_pair:
                        st.dma_start(out=out[i_img, :, :], in_=otile[:, :])
    if DEDUPE_LDW:
        _dedupe_ldweights(nc)
    nc.compile()
    return nc


def _prep_core_inputs(x_b: np.ndarray, k_b: np.ndarray):
    """x_b (16,32,32,64) f32, k_b (3,3,64,128) f32 -> device layouts."""
    np_in = np.float16 if MM_DTYPE == "f16" else np.float32
    xpad = np.zeros((I, HP, WP, C), dtype=np_in)
    xpad[:, 1:H + 1, 1:W + 1, :] = x_b
    # (I, HP, WP, C) -> (I, C, HP, WP) -> (I//2, 2*C, HP*WP)
    xp = np.ascontiguousarray(xpad.transpose(0, 3, 1, 2)).reshape(I // 2, 2 * C, HP * WP)

    kc = k_b.reshape(KD * KD, C, F)                       # (9, 64, 128)
    kdup = np.concatenate([kc, kc], axis=1)               # (9, 128, 128)
    kd = np.ascontiguousarray(kdup.transpose(1, 0, 2)).reshape(128, KD * KD * F)
    if W_DTYPE == "f16" or MM_DTYPE == "f16":
        kd = kd.astype(np.float16)
    return {"xp": xp, "kd": kd}


def kernel(**inputs) -> np.ndarray:
    global _CACHED_NC, LAST_RESULTS
    x = np.asarray(inputs["x"], dtype=np.float32)
    k = np.asarray(inputs["kernel"], dtype=np.float32)

    if _CACHED_NC is None:
        _CACHED_NC = _build_nc()
    nc = _CACHED_NC

    in_maps = [_prep_core_inputs(x[b], k[b]) for b in range(B)]
    res = run_bass_kernel_spmd(nc, in_maps, core_ids=list(range(N_CORES)))
    LAST_RESULTS = res

    outs = []
    for b in range(B):
        o = res.results[b]["out"]                          # (16, 128, 1024)
        o = o.transpose(0, 2, 1).reshape(I, H, W, F)       # (16, 32, 32, 128)
        outs.append(o)
    return np.ascontiguousarray(np.stack(outs, axis=0).astype(np.float32))
